# revision 12
# baseline (speedup 1.0000x reference)
"""GAT (DGL GATConv) over complete per-doc graphs — Trainium2 Bass kernel.

Problem: nn_CompletedSentenceGraph (gnn_message_passing).
  64 docs x 512 sentences, HIDDEN=256, HEADS=4, D=256.
  h = (x @ W).reshape(B,S,H,D)
  el/er = einsum(h, attn_l/attn_r)
  e[b,s,t,h] = leaky_relu(el[s]+er[t], 0.2); alpha = softmax over s
  out = einsum(alpha, h) + bias; return mean over heads  -> [N, 256]

Sharding: data-parallel over docs, 8 docs per core on 8 cores.

Design notes (engine budget per doc, TimelineSim cost model):
  * exp(lrelu(x)) = max(exp(x), exp(0.2 x)); with x = el_s + er_t both exps
    are rank-1:  expe'[s,t] = max(a_s, c_s * m_t)  with a=exp(el),
    c=exp(0.2 el), m=exp(-0.8 er)  (softmax invariant to per-dst scale).
    One fused DVE tensor_scalar per (head, s-chunk).
  * x^T, WLR = W@ALR, and bf16 casts are host-side prep (sharding/layout).
  * One wide 2-bank PSUM pool (tag rotation) serves the er-row matmul, the
    projection [128,4,256] and the aggregation [128,4,256] outputs.
  * Z columns for all (dc, h) accumulate into one tiny PSUM tile via
    1-column matmuls against a constant 4.0 vector (also folds the 1/H
    head-mean); one batched reciprocal per dc.
  * Head combine per dc: 2 ACT scaled-copies (PSUM->SBUF, scale=1/(4Z)) +
    2 DVE scalar_tensor_tensor (mult+add, bias folded into the first) +
    2 Pool adds. GPSIMD must not touch PSUM (walrus rule).
  * Software pipelining: emission order PROJ(d) -> AGG(d-1) -> expe(d) keeps
    the PE queue fed; PE is the bottleneck engine (~10.7us/doc).
"""

from contextlib import ExitStack

import ml_dtypes
import numpy as np

import concourse.mybir as mybir
import concourse.tile as tile
from concourse import bacc
from concourse.bass_utils import run_bass_kernel_spmd

F32 = mybir.dt.float32
BF16 = mybir.dt.bfloat16
AX = mybir.AluOpType
AF = mybir.ActivationFunctionType

NUM_DOCS = 64
S = 512          # sentences per doc
K = 256          # hidden
H = 4            # heads
D = 256          # per-head out feats
N_CORES = 8
DPC = NUM_DOCS // N_CORES  # docs per core
P = 128

SS = S // P      # 4 s-subtiles per doc
KC = K // P      # 2 k-chunks
DC = S // P      # 4 dst chunks

# proj evacuation engine per ss (PSUM -> SBUF, so ACT/DVE only)
EVAC_ENG = ["act", "act", "act", "dve"]


def gat_tile_kernel(tc, xt, w, wlr, bias_m, out):
    nc = tc.nc

    stack = ExitStack()
    with stack:
        consts = stack.enter_context(tc.tile_pool(name="consts", bufs=1))

        # ---------------- setup (once per core) ----------------
        with tc.tile_pool(name="setup_tmp", bufs=1) as setup_tmp:
            w_bf = consts.tile([P, KC, H * D], BF16)
            nc.sync.dma_start(out=w_bf, in_=w.rearrange("(kc p) f -> p kc f", p=P))
            wlr_bf = consts.tile([P, KC, 2 * H], BF16)
            nc.sync.dma_start(out=wlr_bf,
                              in_=wlr.rearrange("(kc p) c -> p kc c", p=P))
            bias_sb = setup_tmp.tile([1, D], F32)
            nc.sync.dma_start(out=bias_sb, in_=bias_m[None, :])
            bias_b = consts.tile([P, D], F32)
            nc.gpsimd.partition_broadcast(bias_b, bias_sb)

        # ---------------- per-doc pipeline ----------------
        with tc.tile_pool(name="xbp", bufs=2) as xbp, \
             tc.tile_pool(name="hp", bufs=2) as hp, \
             tc.tile_pool(name="ep", bufs=2) as ep, \
             tc.tile_pool(name="mp", bufs=2) as mp, \
             tc.tile_pool(name="sp", bufs=3) as sp, \
             tc.tile_pool(name="accp", bufs=2) as accp, \
             tc.tile_pool(name="ps_wide", bufs=2, space="PSUM") as ps_wide, \
             tc.tile_pool(name="ps_pc", bufs=1, space="PSUM") as ps_pc, \
             tc.tile_pool(name="ps_agg", bufs=3, space="PSUM") as ps_agg:

            xt_bf = [None] * DPC

            def x_load(d):
                xt_bf[d] = xbp.tile([P, KC, S], BF16, tag="xtb", name=f"xtb{d}")
                nc.sync.dma_start(
                    out=xt_bf[d],
                    in_=xt[d].rearrange("(kc p) s -> p kc s", p=P))

            state = {}

            def proj(d):
                xb = xt_bf[d]
                # er rows: pt4 [4, 512] in bank 1 of a wide rotation slot
                wt_t = ps_wide.tile([P, H, D], F32, tag="wide", name=f"pt4_{d}")
                pt4 = wt_t[0:4, 0:2, :].rearrange("p a b -> p (a b)")
                for kc in range(KC):
                    nc.tensor.matmul(pt4, lhsT=wlr_bf[:, kc, 0:4],
                                     rhs=xb[:, kc, :],
                                     start=(kc == 0), stop=(kc == KC - 1))
                m_row = sp.tile([4, S], BF16, tag="mrow", name=f"mr{d}")
                nc.scalar.activation(out=m_row, in_=pt4, func=AF.Exp, scale=-0.8)
                m4 = sp.tile([1, H, S], BF16, tag="m4", name=f"m4_{d}")
                # issue from the ACT DGE queue: zero-wait right after m_row,
                # and keeps this latency-critical DMA out of SP's in-order
                # queue (where it would sit behind the out DMAs).
                nc.scalar.dma_start(out=m4, in_=m_row[:, None, :])
                m_b = []
                for h in range(H):
                    mb = mp.tile([P, S], BF16, tag=f"mb{h}", name=f"mb{h}_{d}")
                    m_b.append(mb)
                    nc.gpsimd.partition_broadcast(mb, m4[:, h, :])

                # projection h = x @ W (+ el via WLR), evacuate to SBUF bf16
                pc = ps_pc.tile([P, SS, H], F32, tag="pc", name=f"pc{d}")
                h_aug = []
                for ss in range(SS):
                    wt = ps_wide.tile([P, H, D], F32, tag="wide",
                                      name=f"pw{ss}_{d}")
                    pa = wt[:, 0:2, :].rearrange("p a b -> p (a b)")
                    pb = wt[:, 2:4, :].rearrange("p a b -> p (a b)")
                    for kc in range(KC):
                        lt = xb[:, kc, ss * P:(ss + 1) * P]
                        st = (kc == 0)
                        sp_ = (kc == KC - 1)
                        nc.tensor.matmul(pa, lhsT=lt, rhs=w_bf[:, kc, 0:512],
                                         start=st, stop=sp_)
                        nc.tensor.matmul(pb, lhsT=lt, rhs=w_bf[:, kc, 512:1024],
                                         start=st, stop=sp_)
                        nc.tensor.matmul(pc[:, ss, :], lhsT=lt,
                                         rhs=wlr_bf[:, kc, 4:8],
                                         start=st, stop=sp_)
                    ha = hp.tile([P, H, D + 1], BF16, tag=f"ha{ss}",
                                 name=f"ha{ss}_{d}")
                    h_aug.append(ha)
                    nc.gpsimd.memset(ha[:, :, D:D + 1], 4.0)
                    if EVAC_ENG[ss] == "act":
                        nc.scalar.copy(out=ha[:, :, 0:D], in_=wt)
                    else:
                        nc.vector.tensor_copy(out=ha[:, :, 0:D], in_=wt)

                # a = exp(el), c = exp(0.2 el): [128, SS, 4] f32
                a_bf = sp.tile([P, SS, H], F32, tag="abf", name=f"a{d}")
                c_bf = sp.tile([P, SS, H], F32, tag="cbf", name=f"c{d}")
                nc.scalar.activation(out=a_bf, in_=pc, func=AF.Exp)
                nc.scalar.activation(out=c_bf, in_=pc, func=AF.Exp, scale=0.2)
                state[d] = (m_b, a_bf, c_bf, h_aug)

            def expe_phase(d):
                m_b, a_bf, c_bf, h_aug = state[d]
                expe = []
                for h in range(H):
                    eh = ep.tile([P, SS, S], BF16, tag=f"e{h}", name=f"e{h}_{d}")
                    expe.append(eh)
                    for ss in range(SS):
                        nc.vector.tensor_scalar(
                            out=eh[:, ss, :],
                            in0=m_b[h],
                            scalar1=c_bf[:, ss, h:h + 1],
                            scalar2=a_bf[:, ss, h:h + 1],
                            op0=AX.mult, op1=AX.max)
                state[d] = (expe, h_aug)

            def agg(d):
                expe, h_aug = state[d]
                for dc in range(DC):
                    pu = []
                    for h in range(H):
                        p_h = ps_agg.tile([P, D + 1], F32, tag="pu",
                                          name=f"pu{dc}_{h}_{d}")
                        pu.append(p_h)
                        for sc in range(SS):
                            nc.tensor.matmul(
                                p_h, lhsT=expe[h][:, sc, dc * P:(dc + 1) * P],
                                rhs=h_aug[sc][:, h, :],
                                start=(sc == 0), stop=(sc == SS - 1))
                    # combine: out = sum_h pu_h / (4 Z_h) + bias_mean
                    # (col 256 of each pu is 4 Z_h via the 4.0 rhs column)
                    rz = sp.tile([P, H], F32, tag="rz", name=f"rz{dc}_{d}")
                    for h in range(H):
                        nc.vector.reciprocal(out=rz[:, h:h + 1],
                                             in_=pu[h][:, D:D + 1])
                    t0 = accp.tile([P, D], F32, tag="t0", name=f"t0_{dc}_{d}")
                    nc.scalar.activation(out=t0, in_=pu[0][:, 0:D], func=AF.Copy,
                                         scale=rz[:, 0:1])
                    acc1 = accp.tile([P, D], F32, tag="a1", name=f"a1_{dc}_{d}")
                    nc.vector.scalar_tensor_tensor(
                        out=acc1, in0=pu[1][:, 0:D], scalar=rz[:, 1:2],
                        in1=bias_b, op0=AX.mult, op1=AX.add)
                    t2 = accp.tile([P, D], F32, tag="t2", name=f"t2_{dc}_{d}")
                    nc.scalar.activation(out=t2, in_=pu[2][:, 0:D], func=AF.Copy,
                                         scale=rz[:, 2:3])
                    acc3 = accp.tile([P, D], F32, tag="a3", name=f"a3_{dc}_{d}")
                    nc.vector.scalar_tensor_tensor(
                        out=acc3, in0=pu[3][:, 0:D], scalar=rz[:, 3:4],
                        in1=acc1, op0=AX.mult, op1=AX.add)
                    s02 = accp.tile([P, D], F32, tag="s02", name=f"s02_{dc}_{d}")
                    nc.gpsimd.tensor_tensor(out=s02, in0=t0, in1=t2, op=AX.add)
                    outc = accp.tile([P, D], F32, tag="oc", name=f"oc_{dc}_{d}")
                    nc.gpsimd.tensor_tensor(out=outc, in0=acc3, in1=s02,
                                            op=AX.add)
                    nc.sync.dma_start(
                        out=out[d * S + dc * P:d * S + (dc + 1) * P, :],
                        in_=outc)
                del state[d]

            # emission order per iteration: agg(i-1) first so its combines
            # lead the ACT/DVE queues (freeing agg PSUM promptly), then
            # proj(i) to keep PE fed, then expe(i) at the DVE queue tail.
            x_load(0)
            for i in range(DPC + 1):
                if i + 1 < DPC:
                    x_load(i + 1)
                if i >= 1:
                    agg(i - 1)
                if i < DPC:
                    proj(i)
                    expe_phase(i)


_NC_CACHE = None


def build_nc():
    global _NC_CACHE
    if _NC_CACHE is not None:
        return _NC_CACHE
    nc = bacc.Bacc("TRN2", target_bir_lowering=False, debug=False,
                   num_devices=N_CORES)
    xt = nc.dram_tensor("xt", [DPC, K, S], BF16, kind="ExternalInput")
    w = nc.dram_tensor("w", [K, H * D], BF16, kind="ExternalInput")
    wlr = nc.dram_tensor("wlr", [K, 2 * H], BF16, kind="ExternalInput")
    bias_m = nc.dram_tensor("bias_m", [D], F32, kind="ExternalInput")
    out = nc.dram_tensor("out", [DPC * S, K], F32, kind="ExternalOutput")
    with tile.TileContext(nc) as tc:
        gat_tile_kernel(tc, xt.ap(), w.ap(), wlr.ap(), bias_m.ap(), out.ap())
    nc.compile()
    _NC_CACHE = nc
    return nc


def host_prep(sent_feature, W, attn_l, attn_r, bias):
    """Host-side sharding/layout prep: per-core transposed bf16 x, fused WLR
    (cols 0:4 = attn_r, 4:8 = attn_l), head-mean bias."""
    x = np.asarray(sent_feature, dtype=np.float32)
    W = np.asarray(W, dtype=np.float32)
    al = np.asarray(attn_l, dtype=np.float32)
    ar = np.asarray(attn_r, dtype=np.float32)
    bias = np.asarray(bias, dtype=np.float32)

    w4 = W.reshape(K, H, D)
    wlr = np.concatenate([
        np.einsum("khd,hd->kh", w4, ar),
        np.einsum("khd,hd->kh", w4, al),
    ], axis=1).astype(ml_dtypes.bfloat16)  # [256, 8]
    bias_m = bias.reshape(H, D).mean(axis=0).astype(np.float32)
    w_bf = W.astype(ml_dtypes.bfloat16)

    xts = []
    rows = DPC * S
    for c in range(N_CORES):
        xc = x[c * rows:(c + 1) * rows].reshape(DPC, S, K)
        xts.append(np.ascontiguousarray(
            xc.transpose(0, 2, 1)).astype(ml_dtypes.bfloat16))
    return xts, w_bf, wlr, bias_m


def kernel(sent_feature, W, attn_l, attn_r, bias, num_docs=NUM_DOCS, **_unused):
    xts, w_bf, wlr, bias_m = host_prep(sent_feature, W, attn_l, attn_r, bias)
    nc = build_nc()
    in_maps = []
    for c in range(N_CORES):
        in_maps.append({
            "xt": xts[c], "w": w_bf, "wlr": wlr, "bias_m": bias_m,
        })
    res = run_bass_kernel_spmd(nc, in_maps, core_ids=list(range(N_CORES)))
    out = np.concatenate([res.results[c]["out"] for c in range(N_CORES)], axis=0)
    return out.astype(np.float32)


# revision 14
# speedup vs baseline: 1.0115x; 1.0115x over previous
"""GAT (DGL GATConv) over complete per-doc graphs — Trainium2 Bass kernel.

Problem: nn_CompletedSentenceGraph (gnn_message_passing).
  64 docs x 512 sentences, HIDDEN=256, HEADS=4, D=256.
  h = (x @ W).reshape(B,S,H,D)
  el/er = einsum(h, attn_l/attn_r)
  e[b,s,t,h] = leaky_relu(el[s]+er[t], 0.2); alpha = softmax over s
  out = einsum(alpha, h) + bias; return mean over heads  -> [N, 256]

Sharding: data-parallel over docs, 8 docs per core on 8 cores.

Design notes (engine budget per doc, TimelineSim cost model):
  * exp(lrelu(x)) = max(exp(x), exp(0.2 x)); with x = el_s + er_t both exps
    are rank-1:  expe'[s,t] = max(a_s, c_s * m_t)  with a=exp(el),
    c=exp(0.2 el), m=exp(-0.8 er)  (softmax invariant to per-dst scale).
    One fused DVE tensor_scalar per (head, s-chunk).
  * x^T, WLR = W@ALR, and bf16 casts are host-side prep (sharding/layout).
  * One wide 2-bank PSUM pool (tag rotation) serves the er-row matmul, the
    projection [128,4,256] and the aggregation [128,4,256] outputs.
  * Z columns for all (dc, h) accumulate into one tiny PSUM tile via
    1-column matmuls against a constant 4.0 vector (also folds the 1/H
    head-mean); one batched reciprocal per dc.
  * Head combine per dc: 2 ACT scaled-copies (PSUM->SBUF, scale=1/(4Z)) +
    2 DVE scalar_tensor_tensor (mult+add, bias folded into the first) +
    2 Pool adds. GPSIMD must not touch PSUM (walrus rule).
  * Software pipelining: emission order PROJ(d) -> AGG(d-1) -> expe(d) keeps
    the PE queue fed; PE is the bottleneck engine (~10.7us/doc).
"""

from contextlib import ExitStack

import ml_dtypes
import numpy as np

import concourse.mybir as mybir
import concourse.tile as tile
from concourse import bacc
from concourse.bass_utils import run_bass_kernel_spmd

F32 = mybir.dt.float32
BF16 = mybir.dt.bfloat16
AX = mybir.AluOpType
AF = mybir.ActivationFunctionType

NUM_DOCS = 64
S = 512          # sentences per doc
K = 256          # hidden
H = 4            # heads
D = 256          # per-head out feats
N_CORES = 8
DPC = NUM_DOCS // N_CORES  # docs per core
P = 128

SS = S // P      # 4 s-subtiles per doc
KC = K // P      # 2 k-chunks
DC = S // P      # 4 dst chunks

# proj evacuation engine per ss (PSUM -> SBUF, so ACT/DVE only)
EVAC_ENG = ["act", "act", "act", "dve"]


def gat_tile_kernel(tc, xt, w, wlr, bias_m, out):
    nc = tc.nc

    stack = ExitStack()
    with stack:
        consts = stack.enter_context(tc.tile_pool(name="consts", bufs=1))

        # ---------------- setup (once per core) ----------------
        with tc.tile_pool(name="setup_tmp", bufs=1) as setup_tmp:
            # setup DMAs go through the ACT DGE queue so the first x-load
            # (SP queue) is not delayed behind them at startup
            w_bf = consts.tile([P, KC, H * D], BF16)
            nc.scalar.dma_start(out=w_bf,
                                in_=w.rearrange("(kc p) f -> p kc f", p=P))
            wlr_bf = consts.tile([P, KC, 2 * H], BF16)
            nc.scalar.dma_start(out=wlr_bf,
                                in_=wlr.rearrange("(kc p) c -> p kc c", p=P))
            bias_sb = setup_tmp.tile([1, D], F32)
            nc.scalar.dma_start(out=bias_sb, in_=bias_m[None, :])
            bias_b = consts.tile([P, D], F32)
            nc.gpsimd.partition_broadcast(bias_b, bias_sb)

        # ---------------- per-doc pipeline ----------------
        with tc.tile_pool(name="xbp", bufs=2) as xbp, \
             tc.tile_pool(name="hp", bufs=2) as hp, \
             tc.tile_pool(name="ep", bufs=2) as ep, \
             tc.tile_pool(name="mp", bufs=2) as mp, \
             tc.tile_pool(name="sp", bufs=3) as sp, \
             tc.tile_pool(name="accp", bufs=2) as accp, \
             tc.tile_pool(name="ps_wide", bufs=2, space="PSUM") as ps_wide, \
             tc.tile_pool(name="ps_pc", bufs=1, space="PSUM") as ps_pc, \
             tc.tile_pool(name="ps_agg", bufs=3, space="PSUM") as ps_agg:

            xt_bf = [None] * DPC

            def x_load(d):
                xt_bf[d] = xbp.tile([P, KC, S], BF16, tag="xtb", name=f"xtb{d}")
                nc.sync.dma_start(
                    out=xt_bf[d],
                    in_=xt[d].rearrange("(kc p) s -> p kc s", p=P))

            state = {}

            def proj(d):
                xb = xt_bf[d]
                # er rows: pt4 [4, 512] in bank 1 of a wide rotation slot
                wt_t = ps_wide.tile([P, H, D], F32, tag="wide", name=f"pt4_{d}")
                pt4 = wt_t[0:4, 0:2, :].rearrange("p a b -> p (a b)")
                for kc in range(KC):
                    nc.tensor.matmul(pt4, lhsT=wlr_bf[:, kc, 0:4],
                                     rhs=xb[:, kc, :],
                                     start=(kc == 0), stop=(kc == KC - 1))
                m_row = sp.tile([4, S], BF16, tag="mrow", name=f"mr{d}")
                nc.scalar.activation(out=m_row, in_=pt4, func=AF.Exp, scale=-0.8)
                m4 = sp.tile([1, H, S], BF16, tag="m4", name=f"m4_{d}")
                # issue from the ACT DGE queue: zero-wait right after m_row,
                # and keeps this latency-critical DMA out of SP's in-order
                # queue (where it would sit behind the out DMAs).
                nc.scalar.dma_start(out=m4, in_=m_row[:, None, :])
                m_b = []
                for h in range(H):
                    mb = mp.tile([P, S], BF16, tag=f"mb{h}", name=f"mb{h}_{d}")
                    m_b.append(mb)
                    nc.gpsimd.partition_broadcast(mb, m4[:, h, :])

                # projection h = x @ W (+ el via WLR), evacuate to SBUF bf16
                pc = ps_pc.tile([P, SS, H], F32, tag="pc", name=f"pc{d}")
                h_aug = []
                for ss in range(SS):
                    wt = ps_wide.tile([P, H, D], F32, tag="wide",
                                      name=f"pw{ss}_{d}")
                    pa = wt[:, 0:2, :].rearrange("p a b -> p (a b)")
                    pb = wt[:, 2:4, :].rearrange("p a b -> p (a b)")
                    for kc in range(KC):
                        lt = xb[:, kc, ss * P:(ss + 1) * P]
                        st = (kc == 0)
                        sp_ = (kc == KC - 1)
                        nc.tensor.matmul(pa, lhsT=lt, rhs=w_bf[:, kc, 0:512],
                                         start=st, stop=sp_)
                        nc.tensor.matmul(pb, lhsT=lt, rhs=w_bf[:, kc, 512:1024],
                                         start=st, stop=sp_)
                        nc.tensor.matmul(pc[:, ss, :], lhsT=lt,
                                         rhs=wlr_bf[:, kc, 4:8],
                                         start=st, stop=sp_)
                    ha = hp.tile([P, H, D + 1], BF16, tag=f"ha{ss}",
                                 name=f"ha{ss}_{d}")
                    h_aug.append(ha)
                    nc.gpsimd.memset(ha[:, :, D:D + 1], 4.0)
                    if EVAC_ENG[ss] == "act":
                        nc.scalar.copy(out=ha[:, :, 0:D], in_=wt)
                    else:
                        nc.vector.tensor_copy(out=ha[:, :, 0:D], in_=wt)

                # a = exp(el), c = exp(0.2 el): [128, SS, 4] f32
                a_bf = sp.tile([P, SS, H], F32, tag="abf", name=f"a{d}")
                c_bf = sp.tile([P, SS, H], F32, tag="cbf", name=f"c{d}")
                nc.scalar.activation(out=a_bf, in_=pc, func=AF.Exp)
                nc.scalar.activation(out=c_bf, in_=pc, func=AF.Exp, scale=0.2)
                state[d] = (m_b, a_bf, c_bf, h_aug)

            def expe_phase(d):
                m_b, a_bf, c_bf, h_aug = state[d]
                expe = []
                for h in range(H):
                    eh = ep.tile([P, SS, S], BF16, tag=f"e{h}", name=f"e{h}_{d}")
                    expe.append(eh)
                    for ss in range(SS):
                        nc.vector.tensor_scalar(
                            out=eh[:, ss, :],
                            in0=m_b[h],
                            scalar1=c_bf[:, ss, h:h + 1],
                            scalar2=a_bf[:, ss, h:h + 1],
                            op0=AX.mult, op1=AX.max)
                state[d] = (expe, h_aug)

            def agg(d):
                expe, h_aug = state[d]
                for dc in range(DC):
                    pu = []
                    for h in range(H):
                        p_h = ps_agg.tile([P, D + 1], F32, tag="pu",
                                          name=f"pu{dc}_{h}_{d}")
                        pu.append(p_h)
                        for sc in range(SS):
                            nc.tensor.matmul(
                                p_h, lhsT=expe[h][:, sc, dc * P:(dc + 1) * P],
                                rhs=h_aug[sc][:, h, :],
                                start=(sc == 0), stop=(sc == SS - 1))
                    # combine: out = sum_h pu_h / (4 Z_h) + bias_mean
                    # (col 256 of each pu is 4 Z_h via the 4.0 rhs column)
                    rz = sp.tile([P, H], F32, tag="rz", name=f"rz{dc}_{d}")
                    for h in range(H):
                        nc.vector.reciprocal(out=rz[:, h:h + 1],
                                             in_=pu[h][:, D:D + 1])
                    t0 = accp.tile([P, D], F32, tag="t0", name=f"t0_{dc}_{d}")
                    nc.scalar.activation(out=t0, in_=pu[0][:, 0:D], func=AF.Copy,
                                         scale=rz[:, 0:1])
                    acc1 = accp.tile([P, D], F32, tag="a1", name=f"a1_{dc}_{d}")
                    nc.vector.scalar_tensor_tensor(
                        out=acc1, in0=pu[1][:, 0:D], scalar=rz[:, 1:2],
                        in1=bias_b, op0=AX.mult, op1=AX.add)
                    t2 = accp.tile([P, D], F32, tag="t2", name=f"t2_{dc}_{d}")
                    nc.scalar.activation(out=t2, in_=pu[2][:, 0:D], func=AF.Copy,
                                         scale=rz[:, 2:3])
                    acc3 = accp.tile([P, D], F32, tag="a3", name=f"a3_{dc}_{d}")
                    nc.vector.scalar_tensor_tensor(
                        out=acc3, in0=pu[3][:, 0:D], scalar=rz[:, 3:4],
                        in1=acc1, op0=AX.mult, op1=AX.add)
                    s02 = accp.tile([P, D], F32, tag="s02", name=f"s02_{dc}_{d}")
                    nc.gpsimd.tensor_tensor(out=s02, in0=t0, in1=t2, op=AX.add)
                    outc = accp.tile([P, D], F32, tag="oc", name=f"oc_{dc}_{d}")
                    nc.gpsimd.tensor_tensor(out=outc, in0=acc3, in1=s02,
                                            op=AX.add)
                    nc.sync.dma_start(
                        out=out[d * S + dc * P:d * S + (dc + 1) * P, :],
                        in_=outc)
                del state[d]

            x_load(0)
            for i in range(DPC + 1):
                if i < DPC:
                    if i + 1 < DPC:
                        x_load(i + 1)
                    proj(i)
                if i >= 1:
                    agg(i - 1)
                if i < DPC:
                    expe_phase(i)


_NC_CACHE = None


def build_nc():
    global _NC_CACHE
    if _NC_CACHE is not None:
        return _NC_CACHE
    nc = bacc.Bacc("TRN2", target_bir_lowering=False, debug=False,
                   num_devices=N_CORES)
    xt = nc.dram_tensor("xt", [DPC, K, S], BF16, kind="ExternalInput")
    w = nc.dram_tensor("w", [K, H * D], BF16, kind="ExternalInput")
    wlr = nc.dram_tensor("wlr", [K, 2 * H], BF16, kind="ExternalInput")
    bias_m = nc.dram_tensor("bias_m", [D], F32, kind="ExternalInput")
    out = nc.dram_tensor("out", [DPC * S, K], F32, kind="ExternalOutput")
    with tile.TileContext(nc) as tc:
        gat_tile_kernel(tc, xt.ap(), w.ap(), wlr.ap(), bias_m.ap(), out.ap())
    nc.compile()
    _NC_CACHE = nc
    return nc


def host_prep(sent_feature, W, attn_l, attn_r, bias):
    """Host-side sharding/layout prep: per-core transposed bf16 x, fused WLR
    (cols 0:4 = attn_r, 4:8 = attn_l), head-mean bias."""
    x = np.asarray(sent_feature, dtype=np.float32)
    W = np.asarray(W, dtype=np.float32)
    al = np.asarray(attn_l, dtype=np.float32)
    ar = np.asarray(attn_r, dtype=np.float32)
    bias = np.asarray(bias, dtype=np.float32)

    w4 = W.reshape(K, H, D)
    wlr = np.concatenate([
        np.einsum("khd,hd->kh", w4, ar),
        np.einsum("khd,hd->kh", w4, al),
    ], axis=1).astype(ml_dtypes.bfloat16)  # [256, 8]
    bias_m = bias.reshape(H, D).mean(axis=0).astype(np.float32)
    w_bf = W.astype(ml_dtypes.bfloat16)

    xts = []
    rows = DPC * S
    for c in range(N_CORES):
        xc = x[c * rows:(c + 1) * rows].reshape(DPC, S, K)
        xts.append(np.ascontiguousarray(
            xc.transpose(0, 2, 1)).astype(ml_dtypes.bfloat16))
    return xts, w_bf, wlr, bias_m


def kernel(sent_feature, W, attn_l, attn_r, bias, num_docs=NUM_DOCS, **_unused):
    xts, w_bf, wlr, bias_m = host_prep(sent_feature, W, attn_l, attn_r, bias)
    nc = build_nc()
    in_maps = []
    for c in range(N_CORES):
        in_maps.append({
            "xt": xts[c], "w": w_bf, "wlr": wlr, "bias_m": bias_m,
        })
    res = run_bass_kernel_spmd(nc, in_maps, core_ids=list(range(N_CORES)))
    out = np.concatenate([res.results[c]["out"] for c in range(N_CORES)], axis=0)
    return out.astype(np.float32)


# revision 18
# speedup vs baseline: 1.0182x; 1.0067x over previous
"""GAT (DGL GATConv) over complete per-doc graphs — Trainium2 Bass kernel.

Problem: nn_CompletedSentenceGraph (gnn_message_passing).
  64 docs x 512 sentences, HIDDEN=256, HEADS=4, D=256.
  h = (x @ W).reshape(B,S,H,D)
  el/er = einsum(h, attn_l/attn_r)
  e[b,s,t,h] = leaky_relu(el[s]+er[t], 0.2); alpha = softmax over s
  out = einsum(alpha, h) + bias; return mean over heads  -> [N, 256]

Sharding: data-parallel over docs, 8 docs per core on 8 cores.

Design notes (engine budget per doc, TimelineSim cost model):
  * exp(lrelu(x)) = max(exp(x), exp(0.2 x)); with x = el_s + er_t both exps
    are rank-1:  expe'[s,t] = max(a_s, c_s * m_t)  with a=exp(el),
    c=exp(0.2 el), m=exp(-0.8 er)  (softmax invariant to per-dst scale).
    One fused DVE tensor_scalar per (head, s-chunk).
  * x^T, WLR = W@ALR, and bf16 casts are host-side prep (sharding/layout).
  * One wide 2-bank PSUM pool (tag rotation) serves the er-row matmul, the
    projection [128,4,256] and the aggregation [128,4,256] outputs.
  * Z columns for all (dc, h) accumulate into one tiny PSUM tile via
    1-column matmuls against a constant 4.0 vector (also folds the 1/H
    head-mean); one batched reciprocal per dc.
  * Head combine per dc: 2 ACT scaled-copies (PSUM->SBUF, scale=1/(4Z)) +
    2 DVE scalar_tensor_tensor (mult+add, bias folded into the first) +
    2 Pool adds. GPSIMD must not touch PSUM (walrus rule).
  * Software pipelining: emission order PROJ(d) -> AGG(d-1) -> expe(d) keeps
    the PE queue fed; PE is the bottleneck engine (~10.7us/doc).
"""

from contextlib import ExitStack

import ml_dtypes
import numpy as np

import concourse.mybir as mybir
import concourse.tile as tile
from concourse import bacc
from concourse.bass_utils import run_bass_kernel_spmd

F32 = mybir.dt.float32
BF16 = mybir.dt.bfloat16
AX = mybir.AluOpType
AF = mybir.ActivationFunctionType

NUM_DOCS = 64
S = 512          # sentences per doc
K = 256          # hidden
H = 4            # heads
D = 256          # per-head out feats
N_CORES = 8
DPC = NUM_DOCS // N_CORES  # docs per core
P = 128

SS = S // P      # 4 s-subtiles per doc
KC = K // P      # 2 k-chunks
DC = S // P      # 4 dst chunks

# proj evacuation engine per ss (PSUM -> SBUF, so ACT/DVE only)
EVAC_ENG = ["act", "dve", "act", "dve"]


def gat_tile_kernel(tc, xt, w, wlr, bias_m, out):
    nc = tc.nc

    stack = ExitStack()
    with stack:
        consts = stack.enter_context(tc.tile_pool(name="consts", bufs=1))

        # ---------------- setup (once per core) ----------------
        with tc.tile_pool(name="setup_tmp", bufs=1) as setup_tmp:
            # setup DMAs go through the ACT DGE queue so the first x-load
            # (SP queue) is not delayed behind them at startup
            w_bf = consts.tile([P, KC, H * D], BF16)
            nc.scalar.dma_start(out=w_bf,
                                in_=w.rearrange("(kc p) f -> p kc f", p=P))
            wlr_bf = consts.tile([P, KC, 2 * H], BF16)
            nc.scalar.dma_start(out=wlr_bf,
                                in_=wlr.rearrange("(kc p) c -> p kc c", p=P))
            bias_sb = setup_tmp.tile([1, D], F32)
            nc.scalar.dma_start(out=bias_sb, in_=bias_m[None, :])
            bias_b = consts.tile([P, D], F32)
            nc.gpsimd.partition_broadcast(bias_b, bias_sb)

        # ---------------- per-doc pipeline ----------------
        with tc.tile_pool(name="xbp", bufs=2) as xbp, \
             tc.tile_pool(name="hp", bufs=2) as hp, \
             tc.tile_pool(name="ep", bufs=2) as ep, \
             tc.tile_pool(name="mp", bufs=2) as mp, \
             tc.tile_pool(name="sp", bufs=3) as sp, \
             tc.tile_pool(name="accp", bufs=2) as accp, \
             tc.tile_pool(name="ps_wide", bufs=4, space="PSUM") as ps_wide, \
             tc.tile_pool(name="ps_pc", bufs=1, space="PSUM") as ps_pc, \
             tc.tile_pool(name="ps_agg", bufs=3, space="PSUM") as ps_agg:

            xt_bf = [None] * DPC

            def x_load(d):
                xt_bf[d] = xbp.tile([P, KC, S], BF16, tag="xtb", name=f"xtb{d}")
                nc.sync.dma_start(
                    out=xt_bf[d],
                    in_=xt[d].rearrange("(kc p) s -> p kc s", p=P))

            state = {}

            def proj(d):
                xb = xt_bf[d]
                # er rows: pt4 [4, 512] in one single-bank rotation slot
                wt_t = ps_wide.tile([P, 2, D], F32, tag="pab", name=f"pt4_{d}")
                pt4 = wt_t[0:4, :, :].rearrange("p a b -> p (a b)")
                for kc in range(KC):
                    nc.tensor.matmul(pt4, lhsT=wlr_bf[:, kc, 0:4],
                                     rhs=xb[:, kc, :],
                                     start=(kc == 0), stop=(kc == KC - 1))
                m_row = sp.tile([4, S], BF16, tag="mrow", name=f"mr{d}")
                nc.scalar.activation(out=m_row, in_=pt4, func=AF.Exp, scale=-0.8)
                m4 = sp.tile([1, H, S], BF16, tag="m4", name=f"m4_{d}")
                # issue from the ACT DGE queue: zero-wait right after m_row,
                # and keeps this latency-critical DMA out of SP's in-order
                # queue (where it would sit behind the out DMAs).
                nc.scalar.dma_start(out=m4, in_=m_row[:, None, :])
                m_b = []
                for h in range(H):
                    mb = mp.tile([P, S], BF16, tag=f"mb{h}", name=f"mb{h}_{d}")
                    m_b.append(mb)
                    nc.gpsimd.partition_broadcast(mb, m4[:, h, :])

                # projection h = x @ W (+ el via WLR), evacuate to SBUF bf16
                pc = ps_pc.tile([P, SS, H], F32, tag="pc", name=f"pc{d}")
                h_aug = []
                for ss in range(SS):
                    wta = ps_wide.tile([P, 2, D], F32, tag="pab",
                                       name=f"pwa{ss}_{d}")
                    wtb = ps_wide.tile([P, 2, D], F32, tag="pab",
                                       name=f"pwb{ss}_{d}")
                    pa = wta.rearrange("p a b -> p (a b)")
                    pb = wtb.rearrange("p a b -> p (a b)")
                    lt = [xb[:, kc, ss * P:(ss + 1) * P] for kc in range(KC)]
                    # pa's group stops first so its evacuation overlaps the
                    # pb matmuls (hides PSUM-drain latency from the PE)
                    nc.tensor.matmul(pa, lhsT=lt[0], rhs=w_bf[:, 0, 0:512],
                                     start=True, stop=False)
                    nc.tensor.matmul(pa, lhsT=lt[1], rhs=w_bf[:, 1, 0:512],
                                     start=False, stop=True)
                    nc.tensor.matmul(pb, lhsT=lt[0], rhs=w_bf[:, 0, 512:1024],
                                     start=True, stop=False)
                    nc.tensor.matmul(pb, lhsT=lt[1], rhs=w_bf[:, 1, 512:1024],
                                     start=False, stop=True)
                    nc.tensor.matmul(pc[:, ss, :], lhsT=lt[0],
                                     rhs=wlr_bf[:, 0, 4:8],
                                     start=True, stop=False)
                    nc.tensor.matmul(pc[:, ss, :], lhsT=lt[1],
                                     rhs=wlr_bf[:, 1, 4:8],
                                     start=False, stop=True)
                    ha = hp.tile([P, H, D + 1], BF16, tag=f"ha{ss}",
                                 name=f"ha{ss}_{d}")
                    h_aug.append(ha)
                    nc.gpsimd.memset(ha[:, :, D:D + 1], 4.0)
                    nc.scalar.copy(out=ha[:, 0:2, 0:D], in_=wta)
                    if EVAC_ENG[ss] == "act":
                        nc.scalar.copy(out=ha[:, 2:4, 0:D], in_=wtb)
                    else:
                        nc.vector.tensor_copy(out=ha[:, 2:4, 0:D], in_=wtb)

                # a = exp(el), c = exp(0.2 el): [128, SS, 4] f32
                a_bf = sp.tile([P, SS, H], F32, tag="abf", name=f"a{d}")
                c_bf = sp.tile([P, SS, H], F32, tag="cbf", name=f"c{d}")
                nc.scalar.activation(out=a_bf, in_=pc, func=AF.Exp)
                nc.scalar.activation(out=c_bf, in_=pc, func=AF.Exp, scale=0.2)
                state[d] = (m_b, a_bf, c_bf, h_aug)

            def expe_phase(d):
                m_b, a_bf, c_bf, h_aug = state[d]
                expe = []
                for h in range(H):
                    eh = ep.tile([P, SS, S], BF16, tag=f"e{h}", name=f"e{h}_{d}")
                    expe.append(eh)
                    for ss in range(SS):
                        nc.vector.tensor_scalar(
                            out=eh[:, ss, :],
                            in0=m_b[h],
                            scalar1=c_bf[:, ss, h:h + 1],
                            scalar2=a_bf[:, ss, h:h + 1],
                            op0=AX.mult, op1=AX.max)
                state[d] = (expe, h_aug)

            def agg(d):
                expe, h_aug = state[d]
                for dc in range(DC):
                    pu = []
                    for h in range(H):
                        p_h = ps_agg.tile([P, D + 1], F32, tag="pu",
                                          name=f"pu{dc}_{h}_{d}")
                        pu.append(p_h)
                        for sc in range(SS):
                            nc.tensor.matmul(
                                p_h, lhsT=expe[h][:, sc, dc * P:(dc + 1) * P],
                                rhs=h_aug[sc][:, h, :],
                                start=(sc == 0), stop=(sc == SS - 1))
                    # combine: out = sum_h pu_h / (4 Z_h) + bias_mean
                    # (col 256 of each pu is 4 Z_h via the 4.0 rhs column)
                    rz = sp.tile([P, H], F32, tag="rz", name=f"rz{dc}_{d}")
                    for h in range(H):
                        nc.vector.reciprocal(out=rz[:, h:h + 1],
                                             in_=pu[h][:, D:D + 1])
                    t0 = accp.tile([P, D], F32, tag="t0", name=f"t0_{dc}_{d}")
                    nc.scalar.activation(out=t0, in_=pu[0][:, 0:D], func=AF.Copy,
                                         scale=rz[:, 0:1])
                    acc1 = accp.tile([P, D], F32, tag="a1", name=f"a1_{dc}_{d}")
                    nc.vector.scalar_tensor_tensor(
                        out=acc1, in0=pu[1][:, 0:D], scalar=rz[:, 1:2],
                        in1=bias_b, op0=AX.mult, op1=AX.add)
                    t2 = accp.tile([P, D], F32, tag="t2", name=f"t2_{dc}_{d}")
                    nc.scalar.activation(out=t2, in_=pu[2][:, 0:D], func=AF.Copy,
                                         scale=rz[:, 2:3])
                    acc3 = accp.tile([P, D], F32, tag="a3", name=f"a3_{dc}_{d}")
                    nc.vector.scalar_tensor_tensor(
                        out=acc3, in0=pu[3][:, 0:D], scalar=rz[:, 3:4],
                        in1=acc1, op0=AX.mult, op1=AX.add)
                    s02 = accp.tile([P, D], F32, tag="s02", name=f"s02_{dc}_{d}")
                    nc.gpsimd.tensor_tensor(out=s02, in0=t0, in1=t2, op=AX.add)
                    outc = accp.tile([P, D], F32, tag="oc", name=f"oc_{dc}_{d}")
                    nc.gpsimd.tensor_tensor(out=outc, in0=acc3, in1=s02,
                                            op=AX.add)
                    nc.sync.dma_start(
                        out=out[d * S + dc * P:d * S + (dc + 1) * P, :],
                        in_=outc)
                del state[d]

            x_load(0)
            for i in range(DPC + 1):
                if i < DPC:
                    if i + 1 < DPC:
                        x_load(i + 1)
                    proj(i)
                if i >= 1:
                    agg(i - 1)
                if i < DPC:
                    expe_phase(i)


_NC_CACHE = None


def build_nc():
    global _NC_CACHE
    if _NC_CACHE is not None:
        return _NC_CACHE
    nc = bacc.Bacc("TRN2", target_bir_lowering=False, debug=False,
                   num_devices=N_CORES)
    xt = nc.dram_tensor("xt", [DPC, K, S], BF16, kind="ExternalInput")
    w = nc.dram_tensor("w", [K, H * D], BF16, kind="ExternalInput")
    wlr = nc.dram_tensor("wlr", [K, 2 * H], BF16, kind="ExternalInput")
    bias_m = nc.dram_tensor("bias_m", [D], F32, kind="ExternalInput")
    out = nc.dram_tensor("out", [DPC * S, K], F32, kind="ExternalOutput")
    with tile.TileContext(nc) as tc:
        gat_tile_kernel(tc, xt.ap(), w.ap(), wlr.ap(), bias_m.ap(), out.ap())
    nc.compile()
    _NC_CACHE = nc
    return nc


def host_prep(sent_feature, W, attn_l, attn_r, bias):
    """Host-side sharding/layout prep: per-core transposed bf16 x, fused WLR
    (cols 0:4 = attn_r, 4:8 = attn_l), head-mean bias."""
    x = np.asarray(sent_feature, dtype=np.float32)
    W = np.asarray(W, dtype=np.float32)
    al = np.asarray(attn_l, dtype=np.float32)
    ar = np.asarray(attn_r, dtype=np.float32)
    bias = np.asarray(bias, dtype=np.float32)

    w4 = W.reshape(K, H, D)
    wlr = np.concatenate([
        np.einsum("khd,hd->kh", w4, ar),
        np.einsum("khd,hd->kh", w4, al),
    ], axis=1).astype(ml_dtypes.bfloat16)  # [256, 8]
    bias_m = bias.reshape(H, D).mean(axis=0).astype(np.float32)
    w_bf = W.astype(ml_dtypes.bfloat16)

    xts = []
    rows = DPC * S
    for c in range(N_CORES):
        xc = x[c * rows:(c + 1) * rows].reshape(DPC, S, K)
        xts.append(np.ascontiguousarray(
            xc.transpose(0, 2, 1)).astype(ml_dtypes.bfloat16))
    return xts, w_bf, wlr, bias_m


def kernel(sent_feature, W, attn_l, attn_r, bias, num_docs=NUM_DOCS, **_unused):
    xts, w_bf, wlr, bias_m = host_prep(sent_feature, W, attn_l, attn_r, bias)
    nc = build_nc()
    in_maps = []
    for c in range(N_CORES):
        in_maps.append({
            "xt": xts[c], "w": w_bf, "wlr": wlr, "bias_m": bias_m,
        })
    res = run_bass_kernel_spmd(nc, in_maps, core_ids=list(range(N_CORES)))
    out = np.concatenate([res.results[c]["out"] for c in range(N_CORES)], axis=0)
    return out.astype(np.float32)
